# revision 58
# baseline (speedup 1.0000x reference)
"""Multi-head attention (B=8, N=1024, C=768, H=12, D=64) on 8 TRN2 NeuronCores.

Sharding: pure data parallelism - one batch element per core, no collectives.

v3 (from the v2 software pipeline: ScalarE exp paced, PE filled with
qkv/v/proj work), plus:
  - host pre-tiles every input into its exact SBUF layout so each DMA is a
    single [128 x contiguous] transfer (x split per-ci so the first qk
    accumulation chains start while later chunks stream in);
  - PE warm-up: junk matmuls during the input-DMA window flip the HAM
    clock gate to 8/8 before the real chains start, and the first q/k
    chains are interleaved per-ci so each x chunk feeds two matmuls;
  - div_finish(i-1) is emitted before block i's AV loop; the reciprocal
    runs once over a persistent [33,512] tile (denominators at partitions
    0 and 32) and ONE zero-masked ones-matmul broadcasts both heads'
    reciprocals to the 128-partition bc tile;
  - AV steps and score pairs are emitted in 2-mt batches (halves the
    av<->score stationary alternations, each costing ~100ns of lost
    LDWEIGHTS overlap);
  - e_block(0) becomes block 11's filler so the output projection overlaps
    the final AV/exp; b5 is reordered (q50,k50,k51,q51) so its tail drips
    into block 9;
  - waits stay on MATMULs (move_matmul_waits_to_ldweights disabled), one
    merged SBUF pool + asserts off to shrink the pre/epilogue.

Per-core dataflow (matmuls bf16, accumulation fp32 in PSUM):
  b(hp): qT, kT [d, n] for head pair hp (head-dim on partitions).
  C_a/C_b: v natural [tokens, h*65], 65th col per head = 1.0 (denominator).
  Block (hp, nn): 8 score-pairs S^T[m, n] (two K=64 row-group matmuls run
    concurrently on the PE) -> 1024-wide exp on ScalarE (scale fused) ->
    bf16 P tiles; AV chains for both heads accumulate over mt with the
    NEXT block's score pairs interleaved; denominator lands in row 64 of
    the AV psum; DVE reciprocal -> masked ones-matmul broadcast -> DVE
    multiply writes normalized oT.
  E: yT[o, n] = pwT.T @ oT + bias, with nn=0 overlapping the last block.
Host transposes yT back to [N, C].
"""

import numpy as np

B, N, C, H, D = 8, 1024, 768, 12, 64
SCALE = D ** -0.5
NCORES = 8

CT = C // 128   # 6  c-tiles
HP = H // 2     # 6  head pairs (2 heads of 64 share a 128-partition tile)
NT = N // 512   # 2  n-chunks of 512
MT = N // 128   # 8  m-tiles (keys)
MP = MT // 2    # 4  m-tile pairs (exp granularity)
VW = 65         # v columns per head (64 data + 1 ones)

_CACHE = {}


def _build_nc(dbg=False):
    import concourse.bass as bass
    import concourse.mybir as mybir
    import concourse.tile as tile
    from concourse import bacc

    f32 = mybir.dt.float32
    bf16 = mybir.dt.bfloat16
    AF = mybir.ActivationFunctionType

    nc = bacc.Bacc(
        "TRN2",
        target_bir_lowering=False,
        debug=False,
        enable_asserts=False,
        num_devices=NCORES,
    )

    # all inputs pre-tiled on host to the exact SBUF layout -> each DMA is
    # [128 partitions x contiguous bytes]
    xT_d = nc.dram_tensor("xT", [128, CT * N], bf16, kind="ExternalInput").ap()
    wq_d = nc.dram_tensor("wqT", [128, HP * CT * 128], bf16,
                          kind="ExternalInput").ap()
    wk_d = nc.dram_tensor("wkT", [128, HP * CT * 128], bf16,
                          kind="ExternalInput").ap()
    wv_d = nc.dram_tensor("wvT", [128, CT * C], bf16, kind="ExternalInput").ap()
    pw_d = nc.dram_tensor("pwT", [128, CT * C], bf16, kind="ExternalInput").ap()
    ones_d = nc.dram_tensor("ones2", [33, 128], bf16, kind="ExternalInput").ap()
    pb_d = nc.dram_tensor("pb", [128, CT], f32, kind="ExternalInput").ap()
    out_d = nc.dram_tensor("out", [C, N], bf16, kind="ExternalOutput").ap()

    with tile.TileContext(nc) as tc:
        data = tc.alloc_tile_pool(name="data", bufs=1)
        psp = tc.alloc_tile_pool(name="psp", bufs=1, space="PSUM")
        ptp = data
        small = data

        pb_sb = data.tile([128, CT], f32)
        ones2 = data.tile([33, 128], bf16)

        xTs = data.tile([128, CT * N], bf16)

        # wq/wk hp-major in sbuf: cols = hp*768 + ci*128 + j
        wqs = data.tile([128, HP * CT * 128], bf16)
        wks = data.tile([128, HP * CT * 128], bf16)
        wvs = data.tile([128, CT * C], bf16)
        pws = data.tile([128, CT * C], bf16)

        # dummy tile for PE warm-up (memset first: it gates the warm-up;
        # DVE finishes its preamble earliest)
        wdum = data.tile([128, 640], bf16)
        nc.vector.memset(wdum[:], 0.0)

        # priority order: what the bootstrap needs first; x per-ci so the
        # first accumulation chains start while later chunks stream in.
        nc.sync.dma_start(xTs[:, 0:512], xT_d[:, 0:512])           # x ci0 nn0
        nc.sync.dma_start(wqs[:, 0:768], wq_d[:, 0:768])           # wq hp0
        nc.sync.dma_start(wks[:, 0:768], wk_d[:, 0:768])           # wk hp0
        nc.sync.dma_start(xTs[:, 512:N], xT_d[:, 512:N])           # x ci0 nn1
        for ci in range(1, CT):
            nc.sync.dma_start(xTs[:, ci * N:(ci + 1) * N],
                              xT_d[:, ci * N:(ci + 1) * N])
        nc.sync.dma_start(wks[:, 768:HP * 768], wk_d[:, 768:HP * 768])
        nc.sync.dma_start(wqs[:, 768:HP * 768], wq_d[:, 768:HP * 768])
        nc.sync.dma_start(wvs[:], wv_d[:])
        nc.sync.dma_start(pws[:], pw_d[:])
        nc.sync.dma_start(ones2[:], ones_d[:])
        nc.sync.dma_start(pb_sb[:], pb_d[:])

        qT = data.tile([128, HP * N], bf16)
        kT = data.tile([128, HP * N], bf16)
        va = data.tile([128, MT * H * VW], bf16)
        oT = data.tile([128, HP * N], bf16)

        # persistent denominator/reciprocal tiles: only partitions 0 and
        # 32 are ever rewritten; the rest stay at the memset value so the
        # zero-masked bc matmul never sees NaNs.
        d32b = data.tile([33, 512], f32)
        nc.gpsimd.memset(d32b[:], 1.0)
        r32b = data.tile([33, 512], f32)
        r2b = data.tile([33, 512], bf16)

        # ones columns of v (softmax denominator trick)
        v3 = va[:].rearrange("p (x e) -> p x e", e=VW)
        nc.gpsimd.memset(v3[:, :, 64:65], 1.0)

        def qk_chain(dst, w, hp, nn, tag="out"):
            """One 6-matmul accumulation chain + cast for q or k. Chains
            that complete before block 0's AV may borrow the idle "av"
            psum banks (more slots -> fewer chain-start ring waits)."""
            ps = psp.tile([128, 512], f32, tag=tag, bufs=2, name="ps_qk")
            for ci in range(CT):
                nc.tensor.matmul(
                    ps[:],
                    w[:, hp * 768 + ci * 128: hp * 768 + ci * 128 + 128],
                    xTs[:, ci * N + nn * 512: ci * N + nn * 512 + 512],
                    start=(ci == 0), stop=(ci == CT - 1),
                )
                if ci < CT - 1:
                    yield
            nc.vector.tensor_copy(
                dst[:, hp * N + nn * 512: hp * N + nn * 512 + 512], ps[:])
            yield

        def gen_b_block(hp):
            for dst, w in ((qT, wqs), (kT, wks)):
                for nn in range(NT):
                    yield from qk_chain(dst, w, hp, nn)

        def gen_b5():
            # hp5 in scores-consumption order: q(5,0)+k(5,0) gate block 9's
            # first scores, k(5,1) gates mt>=4, q(5,1) gates block 10 - so
            # the last two chains can drip into block 9's filler slots.
            yield from qk_chain(qT, wqs, 5, 0)
            yield from qk_chain(kT, wks, 5, 0)
            yield from qk_chain(kT, wks, 5, 1)
            yield from qk_chain(qT, wqs, 5, 1)

        def qk0_pair():
            """Interleaved q(0,0)/k(0,0) chains: each x chunk feeds two
            matmuls as it lands, keeping the PE dense during the DMA-paced
            startup."""
            psq = psp.tile([128, 512], f32, tag="out", bufs=2, name="ps_q0")
            psk = psp.tile([128, 512], f32, tag="out", bufs=2, name="ps_k0")
            for ci in range(CT):
                nc.tensor.matmul(
                    psq[:], wqs[:, ci * 128: ci * 128 + 128],
                    xTs[:, ci * N: ci * N + 512],
                    start=(ci == 0), stop=(ci == CT - 1))
                nc.tensor.matmul(
                    psk[:], wks[:, ci * 128: ci * 128 + 128],
                    xTs[:, ci * N: ci * N + 512],
                    start=(ci == 0), stop=(ci == CT - 1))
                if ci < 4:
                    keep_warm(2)   # fill the DMA-paced holes, stay warm
            nc.vector.tensor_copy(qT[:, 0:512], psq[:])
            nc.vector.tensor_copy(kT[:, 0:512], psk[:])

        def c_chain(oc, mt, tag="out"):
            """One v-projection chain + cast for m-chunk mt, out-half oc."""
            ow = 512 if oc == 0 else 256
            nh = ow // 64
            ps = psp.tile([128, 512], f32, tag=tag, bufs=2, name="ps_v")
            for ci in range(CT):
                nc.tensor.matmul(
                    ps[:, :ow],
                    xTs[:, ci * N + mt * 128: ci * N + mt * 128 + 128],
                    wvs[:, ci * C + oc * 512: ci * C + oc * 512 + ow],
                    start=(ci == 0), stop=(ci == CT - 1),
                )
                if ci < CT - 1:
                    yield
            dst3 = va[:, mt * H * VW:(mt + 1) * H * VW].rearrange(
                "p (h e) -> p h e", e=VW)[:, oc * 8: oc * 8 + nh, 0:64]
            src3 = ps[:, :ow].rearrange("p (h d) -> p h d", d=64)
            nc.vector.tensor_copy(dst3, src3)
            yield

        def gen_c_block(oc, mt0, mt1, tag="out"):
            for mt in range(mt0, mt1):
                yield from c_chain(oc, mt, tag=tag)

        def s_block_step(hp, nn, mt, pts):
            """Score pair (both heads of hp) for m-chunk mt, exp into a
            per-mt pt tile (exact dependency granularity for AV)."""
            st = psp.tile([128, 1024], f32, tag="st", bufs=2, name="st")
            for hi in range(2):
                lo = 64 * hi
                nc.tensor.matmul(
                    st[:, hi * 512:(hi + 1) * 512],
                    kT[lo:lo + 64,
                       hp * N + mt * 128: hp * N + mt * 128 + 128],
                    qT[lo:lo + 64,
                       hp * N + nn * 512: hp * N + nn * 512 + 512],
                )
            pt = ptp.tile([128, 1024], bf16, tag="pt", bufs=16, name="pt")
            nc.scalar.activation(
                pt[:].rearrange("p (g x) -> p g x", g=2),
                st[:].rearrange("p (g x) -> p g x", g=2),
                AF.Exp, scale=SCALE)
            pts.append(pt)

        def av_step(av, hp, hi, mt, pts):
            h = 2 * hp + hi
            nc.tensor.matmul(
                av[0:VW, :],
                va[:, mt * H * VW + h * VW: mt * H * VW + h * VW + VW],
                pts[mt][:, hi * 512: hi * 512 + 512],
                start=(mt == 0), stop=(mt == MT - 1),
            )

        def div_start(hp, nn, avs, on_scalar=False):
            """Right after a block's AV chains: copy raw outputs to SBUF
            (frees the av psum slots asap) + reciprocals of the two
            denominator rows batched as [1,1024]. reciprocal_approx_fast
            needs SBUF input."""
            oraw = small.tile([128, 512], bf16, tag="oraw", bufs=6, name="oraw")
            if on_scalar:
                # last block: ScalarE is idle after the final exp, so the
                # denominator copies (which gate recip -> bc -> proj) run
                # there, in parallel with the DVE's oraw copies / e0 adds.
                nc.scalar.activation(d32b[0:1, :], avs[0][64:65, :], AF.Copy)
                nc.scalar.activation(d32b[32:33, :], avs[1][64:65, :],
                                     AF.Copy)
                nc.vector.tensor_copy(oraw[0:64, :], avs[0][0:64, :])
                nc.vector.tensor_copy(oraw[64:128, :], avs[1][0:64, :])
            else:
                # oraw first: it frees the av psum slots asap
                nc.vector.tensor_copy(oraw[0:64, :], avs[0][0:64, :])
                nc.vector.tensor_copy(oraw[64:128, :], avs[1][0:64, :])
                nc.vector.tensor_copy(d32b[0:1, :], avs[0][64:65, :])
                nc.vector.tensor_copy(d32b[32:33, :], avs[1][64:65, :])
            nc.vector.reciprocal_approx_fast(r32b[:], d32b[:])
            nc.vector.tensor_copy(r2b[:], r32b[:])
            return (hp, nn, oraw)

        def div_finish(st):
            """Deferred one block: broadcast recips to partition halves via
            ones-matmuls, then normalize into oT (inputs long ready, so the
            bc matmuls never stall the tensor queue)."""
            hp, nn, oraw = st
            blk = slice(hp * N + nn * 512, hp * N + nn * 512 + 512)
            bc = psp.tile([128, 512], f32, tag="out", bufs=2, name="bc")
            nc.tensor.matmul(bc[:], ones2[:], r2b[:], start=True, stop=True)
            nc.vector.tensor_mul(oT[:, blk], oraw[:], bc[:])

        def e_chain(nn, ot, big=False):
            """One output-projection chain for (n-chunk, out-tile). Chains
            that run after the final exp can borrow the idle score-tile
            psum banks (tag "st"), doubling the chain slots so the ys
            bias-adds never pace the ring."""
            if big:
                yp = psp.tile([128, 1024], f32, tag="st", bufs=2,
                              name="yp2")[:, 0:512]
            else:
                yp = psp.tile([128, 512], f32, tag="out", bufs=2, name="yp")
            for ci in range(CT):
                nc.tensor.matmul(
                    yp[:],
                    pws[:, ci * C + ot * 128: ci * C + ot * 128 + 128],
                    oT[:, ci * N + nn * 512: ci * N + nn * 512 + 512],
                    start=(ci == 0), stop=(ci == CT - 1),
                )
                if ci < CT - 1:
                    yield
            ys = small.tile([128, 512], bf16, tag="ys", bufs=6, name="ys")
            nc.vector.tensor_scalar_add(ys[:], yp[:], pb_sb[:, ot:ot + 1])
            nc.sync.dma_start(
                out_d[ot * 128:(ot + 1) * 128, nn * 512:(nn + 1) * 512],
                ys[:])
            yield

        def gen_e_block(nn, st_from=CT):
            for ot in range(CT):
                yield from e_chain(nn, ot, big=(ot >= st_from))

        # ---------------- pipelined emission ----------------
        blocks = [(hp, nn) for hp in range(HP) for nn in range(NT)]

        # filler: remaining qkv/v tensor work, drip-fed into j-loop steps
        # so the PE stays continuously busy while ScalarE paces on exp.
        import itertools
        filler_gen = [itertools.chain(
            gen_b_block(2), gen_b_block(3), gen_b_block(4),
            gen_c_block(1, 0, 8), gen_b5())]
        pulled = [0]

        def pull(k):
            for _ in range(k):
                try:
                    next(filler_gen[0])
                    pulled[0] += 1
                except StopIteration:
                    return

        # filler units that must be emitted before block i's j-loop:
        # b2<=i3 (24), b3<=i5 (48), b4<=i7 (72), C_b<=i8 (120),
        # b5 q50/k50 <= i9 (132); k51/q51 drip into block 9's pulls.
        DEADLINE = {3: 24, 5: 48, 7: 72, 8: 120, 9: 132}

        # ---- PE warm-up: junk matmuls while the input DMAs stream ------
        warm_ps = psp.tile([128, 512], f32, tag="av", bufs=2, name="warm")

        def keep_warm(k):
            for _ in range(k):
                nc.tensor.matmul(warm_ps[:], wdum[:, 0:128], wdum[:, 128:640],
                                 start=True, stop=True)

        keep_warm(10)

        # ---- bootstrap: earliest possible exp start --------------------
        cur_pts = []
        qk0_pair()
        for mt in range(4):
            s_block_step(0, 0, mt, cur_pts)
            keep_warm(1)
        for _ in qk_chain(kT, wks, 0, 1, tag="av"):
            pass
        for mt in range(4, MT):
            s_block_step(0, 0, mt, cur_pts)
        for _ in qk_chain(qT, wqs, 0, 1, tag="av"):
            pass
        for dst, w in ((qT, wqs), (kT, wks)):       # b1 on the av banks
            for nn in range(NT):
                for _ in qk_chain(dst, w, 1, nn, tag="av"):
                    pass
        for _ in gen_c_block(0, 0, 8, tag="av"):    # C_a on the av banks
            pass

        # ---- steady state ----------------------------------------------
        pending = None
        for i, (hp, nn) in enumerate(blocks):
            need = DEADLINE.get(i, 0)
            while pulled[0] < need:
                pull(1)
            nxt = blocks[i + 1] if i + 1 < len(blocks) else None
            nxt_pts = []
            # finish the PREVIOUS block's normalize first: its recips are
            # long done, so the broadcasts + mul overlap this block's AV
            # instead of queueing behind this block's reciprocal chain.
            if pending is not None:
                div_finish(pending)
                pending = None
            if i == 11:
                # all nn0 oT slabs are normalized; the output projection
                # becomes this block's filler so it overlaps the final AV.
                filler_gen[0] = gen_e_block(0, st_from=4)
            av0 = psp.tile([128, 512], f32, tag="av", bufs=2, name="av0")
            av1 = psp.tile([128, 512], f32, tag="av", bufs=2, name="av1")
            st_div = None
            for mtp in range(MT // 2):
                # 2-mt batches: grouping the AV steps and score pairs halves
                # the av<->score stationary alternations (each costs ~100ns
                # of lost LDWEIGHTS overlap on the PE).
                for mt in (2 * mtp, 2 * mtp + 1):
                    av_step(av0, hp, 0, mt, cur_pts)
                    av_step(av1, hp, 1, mt, cur_pts)
                if mtp == MT // 2 - 1:
                    # AV chains just completed: queue the copy-out before
                    # the trailing scores/filler so the av psum slots and
                    # the reciprocal start as early as possible.
                    st_div = div_start(hp, nn, (av0, av1),
                                       on_scalar=(i == 11))
                if nxt:
                    s_block_step(nxt[0], nxt[1], 2 * mtp, nxt_pts)
                    s_block_step(nxt[0], nxt[1], 2 * mtp + 1, nxt_pts)
                pull(5 if i == 11 else 4)
            pull(2)
            pending = st_div
            cur_pts = nxt_pts
        for _ in filler_gen[0]:   # drain remaining e_block(0) work; this
            pass                  # covers the last reciprocal's latency
        div_finish(pending)
        for _ in gen_e_block(1, st_from=3):
            pass

        psp.release()
        data.release()

    # Keep semaphore waits on the MATMULs instead of migrating them onto
    # their LDWEIGHTS: a wait-carrying LDWEIGHTS cannot be pulled ahead by
    # the PE's reorder window, which costs ~100ns per affected matmul.
    # Extra waits get split into EVENT_SEMAPHORE instructions instead.
    nc.move_matmul_waits_to_ldweights = lambda: None
    nc.compile()
    return nc


def _get_nc():
    if "nc" not in _CACHE:
        _CACHE["nc"] = _build_nc()
    return _CACHE["nc"]


def _prep_in_maps(x, qkv_w, proj_w, proj_b):
    import ml_dtypes

    bf16 = ml_dtypes.bfloat16
    x = np.asarray(x, dtype=np.float32)
    qkv_w = np.asarray(qkv_w, dtype=np.float32)
    proj_w = np.asarray(proj_w, dtype=np.float32)
    proj_b = np.asarray(proj_b, dtype=np.float32)

    def by_hp(wT):  # [C(in), C(out)] -> sbuf layout [128, HP*CT*128]
        w4 = wT.reshape(CT, 128, HP, 128).transpose(1, 2, 0, 3)
        return np.ascontiguousarray(w4.reshape(128, HP * CT * 128)).astype(bf16)

    def by_ci(wT):  # [C(in), C(out)] -> sbuf layout [128, CT*C]
        w3 = wT.reshape(CT, 128, C).transpose(1, 0, 2)
        return np.ascontiguousarray(w3.reshape(128, CT * C)).astype(bf16)

    wq_hp = by_hp(np.ascontiguousarray(qkv_w[0:C].T))         # [in, out] tiled
    wk_hp = by_hp(np.ascontiguousarray(qkv_w[C:2 * C].T))
    wvT = by_ci(np.ascontiguousarray(qkv_w[2 * C:3 * C].T))
    pwT = by_ci(np.ascontiguousarray(proj_w.T))
    pb = np.ascontiguousarray(proj_b.reshape(CT, 128).T)      # [128, CT] f32
    ones2 = np.zeros((33, 128), dtype=np.float32)
    ones2[0, 0:64] = 1.0
    ones2[32, 64:128] = 1.0
    ones2 = ones2.astype(bf16)

    in_maps = []
    for b in range(B):
        # xT sbuf layout [128, CT*N]: col ci*N+n = x[n, ci*128+p]
        xt = np.ascontiguousarray(
            x[b].T.reshape(CT, 128, N).transpose(1, 0, 2).reshape(128, CT * N)
        ).astype(bf16)
        in_maps.append({
            "xT": xt,
            "wqT": wq_hp, "wkT": wk_hp, "wvT": wvT, "pwT": pwT, "pb": pb,
            "ones2": ones2,
        })
    return in_maps


def _run(in_maps, **kwargs):
    from concourse.bass_utils import run_bass_kernel_spmd

    return run_bass_kernel_spmd(_get_nc(), in_maps,
                                core_ids=list(range(NCORES)), **kwargs)


def _gather(res):
    out = np.stack([res.results[b]["out"].T for b in range(B)], axis=0)
    return np.ascontiguousarray(out.astype(np.float32))


def kernel(x, qkv_w, proj_w, proj_b):
    return _gather(_run(_prep_in_maps(x, qkv_w, proj_w, proj_b)))


# revision 59
# speedup vs baseline: 1.0091x; 1.0091x over previous
"""Multi-head attention (B=8, N=1024, C=768, H=12, D=64) on 8 TRN2 NeuronCores.

Sharding: pure data parallelism - one batch element per core, no collectives.

v3 (from the v2 software pipeline: ScalarE exp paced, PE filled with
qkv/v/proj work), plus:
  - host pre-tiles every input into its exact SBUF layout so each DMA is a
    single [128 x contiguous] transfer (x split per-ci so the first qk
    accumulation chains start while later chunks stream in);
  - PE warm-up: junk matmuls during the input-DMA window flip the HAM
    clock gate to 8/8 before the real chains start, and the first q/k
    chains are interleaved per-ci so each x chunk feeds two matmuls;
  - div_finish(i-1) is emitted before block i's AV loop; the reciprocal
    runs once over a persistent [33,512] tile (denominators at partitions
    0 and 32) and ONE zero-masked ones-matmul broadcasts both heads'
    reciprocals to the 128-partition bc tile;
  - AV steps and score pairs are emitted in 2-mt batches (halves the
    av<->score stationary alternations, each costing ~100ns of lost
    LDWEIGHTS overlap);
  - e_block(0) becomes block 11's filler so the output projection overlaps
    the final AV/exp; b5 is reordered (q50,k50,k51,q51) so its tail drips
    into block 9;
  - waits stay on MATMULs (move_matmul_waits_to_ldweights disabled), one
    merged SBUF pool + asserts off to shrink the pre/epilogue.

Per-core dataflow (matmuls bf16, accumulation fp32 in PSUM):
  b(hp): qT, kT [d, n] for head pair hp (head-dim on partitions).
  C_a/C_b: v natural [tokens, h*65], 65th col per head = 1.0 (denominator).
  Block (hp, nn): 8 score-pairs S^T[m, n] (two K=64 row-group matmuls run
    concurrently on the PE) -> 1024-wide exp on ScalarE (scale fused) ->
    bf16 P tiles; AV chains for both heads accumulate over mt with the
    NEXT block's score pairs interleaved; denominator lands in row 64 of
    the AV psum; DVE reciprocal -> masked ones-matmul broadcast -> DVE
    multiply writes normalized oT.
  E: yT[o, n] = pwT.T @ oT + bias, with nn=0 overlapping the last block.
Host transposes yT back to [N, C].
"""

import numpy as np

B, N, C, H, D = 8, 1024, 768, 12, 64
SCALE = D ** -0.5
NCORES = 8

CT = C // 128   # 6  c-tiles
HP = H // 2     # 6  head pairs (2 heads of 64 share a 128-partition tile)
NT = N // 512   # 2  n-chunks of 512
MT = N // 128   # 8  m-tiles (keys)
MP = MT // 2    # 4  m-tile pairs (exp granularity)
VW = 65         # v columns per head (64 data + 1 ones)

_CACHE = {}


def _build_nc(dbg=False):
    import concourse.bass as bass
    import concourse.mybir as mybir
    import concourse.tile as tile
    from concourse import bacc

    f32 = mybir.dt.float32
    bf16 = mybir.dt.bfloat16
    AF = mybir.ActivationFunctionType

    nc = bacc.Bacc(
        "TRN2",
        target_bir_lowering=False,
        debug=False,
        enable_asserts=False,
        num_devices=NCORES,
    )

    # all inputs pre-tiled on host to the exact SBUF layout -> each DMA is
    # [128 partitions x contiguous bytes]
    xT_d = nc.dram_tensor("xT", [128, CT * N], bf16, kind="ExternalInput").ap()
    wq_d = nc.dram_tensor("wqT", [128, HP * CT * 128], bf16,
                          kind="ExternalInput").ap()
    wk_d = nc.dram_tensor("wkT", [128, HP * CT * 128], bf16,
                          kind="ExternalInput").ap()
    wv_d = nc.dram_tensor("wvT", [128, CT * C], bf16, kind="ExternalInput").ap()
    pw_d = nc.dram_tensor("pwT", [128, CT * C], bf16, kind="ExternalInput").ap()
    ones_d = nc.dram_tensor("ones2", [33, 128], bf16, kind="ExternalInput").ap()
    pb_d = nc.dram_tensor("pb", [128, CT], f32, kind="ExternalInput").ap()
    out_d = nc.dram_tensor("out", [C, N], bf16, kind="ExternalOutput").ap()

    with tile.TileContext(nc) as tc:
        data = tc.alloc_tile_pool(name="data", bufs=1)
        psp = tc.alloc_tile_pool(name="psp", bufs=1, space="PSUM")
        ptp = data
        small = data

        pb_sb = data.tile([128, CT], f32)
        ones2 = data.tile([33, 128], bf16)

        xTs = data.tile([128, CT * N], bf16)

        # wq/wk hp-major in sbuf: cols = hp*768 + ci*128 + j
        wqs = data.tile([128, HP * CT * 128], bf16)
        wks = data.tile([128, HP * CT * 128], bf16)
        wvs = data.tile([128, CT * C], bf16)
        pws = data.tile([128, CT * C], bf16)

        # dummy tile for PE warm-up (memset first: it gates the warm-up;
        # DVE finishes its preamble earliest)
        wdum = data.tile([128, 640], bf16)
        nc.vector.memset(wdum[:], 0.0)

        # priority order: what the bootstrap needs first; x per-ci so the
        # first accumulation chains start while later chunks stream in.
        nc.sync.dma_start(xTs[:, 0:512], xT_d[:, 0:512])           # x ci0 nn0
        nc.sync.dma_start(wqs[:, 0:768], wq_d[:, 0:768])           # wq hp0
        nc.sync.dma_start(wks[:, 0:768], wk_d[:, 0:768])           # wk hp0
        nc.sync.dma_start(xTs[:, 512:N], xT_d[:, 512:N])           # x ci0 nn1
        for ci in range(1, CT):
            nc.sync.dma_start(xTs[:, ci * N:(ci + 1) * N],
                              xT_d[:, ci * N:(ci + 1) * N])
        nc.sync.dma_start(wks[:, 768:HP * 768], wk_d[:, 768:HP * 768])
        nc.sync.dma_start(wqs[:, 768:HP * 768], wq_d[:, 768:HP * 768])
        nc.sync.dma_start(wvs[:], wv_d[:])
        nc.sync.dma_start(pws[:], pw_d[:])
        nc.sync.dma_start(ones2[:], ones_d[:])
        nc.sync.dma_start(pb_sb[:], pb_d[:])

        qT = data.tile([128, HP * N], bf16)
        kT = data.tile([128, HP * N], bf16)
        va = data.tile([128, MT * H * VW], bf16)
        oT = data.tile([128, HP * N], bf16)

        # persistent denominator/reciprocal tiles: only partitions 0 and
        # 32 are ever rewritten; the rest stay at the memset value so the
        # zero-masked bc matmul never sees NaNs.
        d32b = data.tile([33, 512], f32)
        nc.gpsimd.memset(d32b[:], 1.0)
        r32b = data.tile([33, 512], f32)
        r2b = data.tile([33, 512], bf16)

        # ones columns of v (softmax denominator trick)
        v3 = va[:].rearrange("p (x e) -> p x e", e=VW)
        nc.gpsimd.memset(v3[:, :, 64:65], 1.0)

        def qk_chain(dst, w, hp, nn, tag="out"):
            """One 6-matmul accumulation chain + cast for q or k. Chains
            that complete before block 0's AV may borrow the idle "av"
            psum banks (more slots -> fewer chain-start ring waits)."""
            ps = psp.tile([128, 512], f32, tag=tag, bufs=2, name="ps_qk")
            for ci in range(CT):
                nc.tensor.matmul(
                    ps[:],
                    w[:, hp * 768 + ci * 128: hp * 768 + ci * 128 + 128],
                    xTs[:, ci * N + nn * 512: ci * N + nn * 512 + 512],
                    start=(ci == 0), stop=(ci == CT - 1),
                )
                if ci < CT - 1:
                    yield
            nc.vector.tensor_copy(
                dst[:, hp * N + nn * 512: hp * N + nn * 512 + 512], ps[:])
            yield

        def gen_b_block(hp):
            for dst, w in ((qT, wqs), (kT, wks)):
                for nn in range(NT):
                    yield from qk_chain(dst, w, hp, nn)

        def gen_b5():
            # hp5 in scores-consumption order: q(5,0)+k(5,0) gate block 9's
            # first scores, k(5,1) gates mt>=4, q(5,1) gates block 10 - so
            # the last two chains can drip into block 9's filler slots.
            yield from qk_chain(qT, wqs, 5, 0)
            yield from qk_chain(kT, wks, 5, 0)
            yield from qk_chain(kT, wks, 5, 1)
            yield from qk_chain(qT, wqs, 5, 1)

        def qk0_pair():
            """Interleaved q(0,0)/k(0,0) chains: each x chunk feeds two
            matmuls as it lands, keeping the PE dense during the DMA-paced
            startup."""
            psq = psp.tile([128, 512], f32, tag="out", bufs=2, name="ps_q0")
            psk = psp.tile([128, 512], f32, tag="out", bufs=2, name="ps_k0")
            for ci in range(CT):
                nc.tensor.matmul(
                    psq[:], wqs[:, ci * 128: ci * 128 + 128],
                    xTs[:, ci * N: ci * N + 512],
                    start=(ci == 0), stop=(ci == CT - 1))
                nc.tensor.matmul(
                    psk[:], wks[:, ci * 128: ci * 128 + 128],
                    xTs[:, ci * N: ci * N + 512],
                    start=(ci == 0), stop=(ci == CT - 1))
                if ci < 4:
                    keep_warm(2)   # fill the DMA-paced holes, stay warm
            nc.vector.tensor_copy(qT[:, 0:512], psq[:])
            nc.vector.tensor_copy(kT[:, 0:512], psk[:])

        def c_chain(oc, mt, tag="out"):
            """One v-projection chain + cast for m-chunk mt, out-half oc."""
            ow = 512 if oc == 0 else 256
            nh = ow // 64
            ps = psp.tile([128, 512], f32, tag=tag, bufs=2, name="ps_v")
            for ci in range(CT):
                nc.tensor.matmul(
                    ps[:, :ow],
                    xTs[:, ci * N + mt * 128: ci * N + mt * 128 + 128],
                    wvs[:, ci * C + oc * 512: ci * C + oc * 512 + ow],
                    start=(ci == 0), stop=(ci == CT - 1),
                )
                if ci < CT - 1:
                    yield
            dst3 = va[:, mt * H * VW:(mt + 1) * H * VW].rearrange(
                "p (h e) -> p h e", e=VW)[:, oc * 8: oc * 8 + nh, 0:64]
            src3 = ps[:, :ow].rearrange("p (h d) -> p h d", d=64)
            nc.vector.tensor_copy(dst3, src3)
            yield

        def gen_c_block(oc, mt0, mt1, tag="out"):
            for mt in range(mt0, mt1):
                yield from c_chain(oc, mt, tag=tag)

        def s_block_step(hp, nn, mt, pts):
            """Score pair (both heads of hp) for m-chunk mt, exp into a
            per-mt pt tile (exact dependency granularity for AV)."""
            st = psp.tile([128, 1024], f32, tag="st", bufs=2, name="st")
            for hi in range(2):
                lo = 64 * hi
                nc.tensor.matmul(
                    st[:, hi * 512:(hi + 1) * 512],
                    kT[lo:lo + 64,
                       hp * N + mt * 128: hp * N + mt * 128 + 128],
                    qT[lo:lo + 64,
                       hp * N + nn * 512: hp * N + nn * 512 + 512],
                )
            pt = ptp.tile([128, 1024], bf16, tag="pt", bufs=16, name="pt")
            nc.scalar.activation(
                pt[:].rearrange("p (g x) -> p g x", g=2),
                st[:].rearrange("p (g x) -> p g x", g=2),
                AF.Exp, scale=SCALE)
            pts.append(pt)

        def av_step(av, hp, hi, mt, pts):
            h = 2 * hp + hi
            nc.tensor.matmul(
                av[0:VW, :],
                va[:, mt * H * VW + h * VW: mt * H * VW + h * VW + VW],
                pts[mt][:, hi * 512: hi * 512 + 512],
                start=(mt == 0), stop=(mt == MT - 1),
            )

        def div_start(hp, nn, avs, on_scalar=False):
            """Right after a block's AV chains: copy raw outputs to SBUF
            (frees the av psum slots asap) + reciprocals of the two
            denominator rows batched as [1,1024]. reciprocal_approx_fast
            needs SBUF input."""
            oraw = small.tile([128, 512], bf16, tag="oraw", bufs=6, name="oraw")
            if on_scalar:
                # last block: ScalarE is idle after the final exp, so the
                # denominator copies (which gate recip -> bc -> proj) run
                # there, in parallel with the DVE's oraw copies / e0 adds.
                nc.scalar.activation(d32b[0:1, :], avs[0][64:65, :], AF.Copy)
                nc.scalar.activation(d32b[32:33, :], avs[1][64:65, :],
                                     AF.Copy)
                nc.vector.tensor_copy(oraw[0:64, :], avs[0][0:64, :])
                nc.vector.tensor_copy(oraw[64:128, :], avs[1][0:64, :])
            else:
                # oraw first: it frees the av psum slots asap
                nc.vector.tensor_copy(oraw[0:64, :], avs[0][0:64, :])
                nc.vector.tensor_copy(oraw[64:128, :], avs[1][0:64, :])
                nc.vector.tensor_copy(d32b[0:1, :], avs[0][64:65, :])
                nc.vector.tensor_copy(d32b[32:33, :], avs[1][64:65, :])
            nc.vector.reciprocal_approx_fast(r32b[:], d32b[:])
            nc.vector.tensor_copy(r2b[:], r32b[:])
            return (hp, nn, oraw)

        def div_finish(st):
            """Deferred one block: broadcast recips to partition halves via
            ones-matmuls, then normalize into oT (inputs long ready, so the
            bc matmuls never stall the tensor queue)."""
            hp, nn, oraw = st
            blk = slice(hp * N + nn * 512, hp * N + nn * 512 + 512)
            bc = psp.tile([128, 512], f32, tag="out", bufs=2, name="bc")
            nc.tensor.matmul(bc[:], ones2[:], r2b[:], start=True, stop=True)
            nc.vector.tensor_mul(oT[:, blk], oraw[:], bc[:])

        def e_chain(nn, ot, big=False):
            """One output-projection chain for (n-chunk, out-tile). Chains
            that run after the final exp can borrow the idle score-tile
            psum banks (tag "st"), doubling the chain slots so the ys
            bias-adds never pace the ring."""
            if big:
                yp = psp.tile([128, 1024], f32, tag="st", bufs=2,
                              name="yp2")[:, 0:512]
            else:
                yp = psp.tile([128, 512], f32, tag="out", bufs=2, name="yp")
            for ci in range(CT):
                nc.tensor.matmul(
                    yp[:],
                    pws[:, ci * C + ot * 128: ci * C + ot * 128 + 128],
                    oT[:, ci * N + nn * 512: ci * N + nn * 512 + 512],
                    start=(ci == 0), stop=(ci == CT - 1),
                )
                if ci < CT - 1:
                    yield
            ys = small.tile([128, 512], bf16, tag="ys", bufs=6, name="ys")
            nc.vector.tensor_scalar_add(ys[:], yp[:], pb_sb[:, ot:ot + 1])
            nc.sync.dma_start(
                out_d[ot * 128:(ot + 1) * 128, nn * 512:(nn + 1) * 512],
                ys[:])
            yield

        def gen_e_block(nn, st_from=CT):
            for ot in range(CT):
                yield from e_chain(nn, ot, big=(ot >= st_from))

        # ---------------- pipelined emission ----------------
        blocks = [(hp, nn) for hp in range(HP) for nn in range(NT)]

        # filler: remaining qkv/v tensor work, drip-fed into j-loop steps
        # so the PE stays continuously busy while ScalarE paces on exp.
        import itertools
        filler_gen = [itertools.chain(
            gen_b_block(2), gen_b_block(3), gen_b_block(4),
            gen_c_block(1, 0, 8), gen_b5())]
        pulled = [0]

        def pull(k):
            for _ in range(k):
                try:
                    next(filler_gen[0])
                    pulled[0] += 1
                except StopIteration:
                    return

        # filler units that must be emitted before block i's j-loop:
        # b2<=i3 (24), b3<=i5 (48), b4<=i7 (72), C_b<=i8 (120),
        # b5 q50/k50 <= i9 (132); k51/q51 drip into block 9's pulls.
        DEADLINE = {3: 24, 5: 48, 7: 72, 8: 120, 9: 132}

        # ---- PE warm-up: junk matmuls while the input DMAs stream ------
        warm_ps = psp.tile([128, 512], f32, tag="av", bufs=2, name="warm")

        def keep_warm(k):
            for _ in range(k):
                nc.tensor.matmul(warm_ps[:], wdum[:, 0:128], wdum[:, 128:640],
                                 start=True, stop=True)

        keep_warm(10)

        # ---- bootstrap: earliest possible exp start --------------------
        cur_pts = []
        qk0_pair()
        for mt in range(4):
            s_block_step(0, 0, mt, cur_pts)
            keep_warm(1)
        for _ in qk_chain(kT, wks, 0, 1):
            pass
        for mt in range(4, MT):
            s_block_step(0, 0, mt, cur_pts)
        for _ in qk_chain(qT, wqs, 0, 1):
            pass
        for _ in gen_b_block(1):
            pass
        for _ in gen_c_block(0, 0, 8):
            pass

        # ---- steady state ----------------------------------------------
        pending = None
        for i, (hp, nn) in enumerate(blocks):
            need = DEADLINE.get(i, 0)
            while pulled[0] < need:
                pull(1)
            nxt = blocks[i + 1] if i + 1 < len(blocks) else None
            nxt_pts = []
            # finish the PREVIOUS block's normalize first: its recips are
            # long done, so the broadcasts + mul overlap this block's AV
            # instead of queueing behind this block's reciprocal chain.
            if pending is not None:
                div_finish(pending)
                pending = None
            if i == 11:
                # all nn0 oT slabs are normalized; the output projection
                # becomes this block's filler so it overlaps the final AV.
                filler_gen[0] = gen_e_block(0, st_from=4)
            av0 = psp.tile([128, 512], f32, tag="av", bufs=2, name="av0")
            av1 = psp.tile([128, 512], f32, tag="av", bufs=2, name="av1")
            st_div = None
            for mtp in range(MT // 2):
                # 2-mt batches: grouping the AV steps and score pairs halves
                # the av<->score stationary alternations (each costs ~100ns
                # of lost LDWEIGHTS overlap on the PE).
                for mt in (2 * mtp, 2 * mtp + 1):
                    av_step(av0, hp, 0, mt, cur_pts)
                    av_step(av1, hp, 1, mt, cur_pts)
                if mtp == MT // 2 - 1:
                    # AV chains just completed: queue the copy-out before
                    # the trailing scores/filler so the av psum slots and
                    # the reciprocal start as early as possible.
                    st_div = div_start(hp, nn, (av0, av1),
                                       on_scalar=(i == 11))
                if nxt:
                    s_block_step(nxt[0], nxt[1], 2 * mtp, nxt_pts)
                    s_block_step(nxt[0], nxt[1], 2 * mtp + 1, nxt_pts)
                pull(5 if i == 11 else 4)
            pull(2)
            pending = st_div
            cur_pts = nxt_pts
        for _ in filler_gen[0]:   # drain remaining e_block(0) work; this
            pass                  # covers the last reciprocal's latency
        div_finish(pending)
        for _ in gen_e_block(1, st_from=3):
            pass

        psp.release()
        data.release()

    # Keep semaphore waits on the MATMULs instead of migrating them onto
    # their LDWEIGHTS: a wait-carrying LDWEIGHTS cannot be pulled ahead by
    # the PE's reorder window, which costs ~100ns per affected matmul.
    # Extra waits get split into EVENT_SEMAPHORE instructions instead.
    nc.move_matmul_waits_to_ldweights = lambda: None
    nc.compile()
    return nc


def _get_nc():
    if "nc" not in _CACHE:
        _CACHE["nc"] = _build_nc()
    return _CACHE["nc"]


def _prep_in_maps(x, qkv_w, proj_w, proj_b):
    import ml_dtypes

    bf16 = ml_dtypes.bfloat16
    x = np.asarray(x, dtype=np.float32)
    qkv_w = np.asarray(qkv_w, dtype=np.float32)
    proj_w = np.asarray(proj_w, dtype=np.float32)
    proj_b = np.asarray(proj_b, dtype=np.float32)

    def by_hp(wT):  # [C(in), C(out)] -> sbuf layout [128, HP*CT*128]
        w4 = wT.reshape(CT, 128, HP, 128).transpose(1, 2, 0, 3)
        return np.ascontiguousarray(w4.reshape(128, HP * CT * 128)).astype(bf16)

    def by_ci(wT):  # [C(in), C(out)] -> sbuf layout [128, CT*C]
        w3 = wT.reshape(CT, 128, C).transpose(1, 0, 2)
        return np.ascontiguousarray(w3.reshape(128, CT * C)).astype(bf16)

    wq_hp = by_hp(np.ascontiguousarray(qkv_w[0:C].T))         # [in, out] tiled
    wk_hp = by_hp(np.ascontiguousarray(qkv_w[C:2 * C].T))
    wvT = by_ci(np.ascontiguousarray(qkv_w[2 * C:3 * C].T))
    pwT = by_ci(np.ascontiguousarray(proj_w.T))
    pb = np.ascontiguousarray(proj_b.reshape(CT, 128).T)      # [128, CT] f32
    ones2 = np.zeros((33, 128), dtype=np.float32)
    ones2[0, 0:64] = 1.0
    ones2[32, 64:128] = 1.0
    ones2 = ones2.astype(bf16)

    in_maps = []
    for b in range(B):
        # xT sbuf layout [128, CT*N]: col ci*N+n = x[n, ci*128+p]
        xt = np.ascontiguousarray(
            x[b].T.reshape(CT, 128, N).transpose(1, 0, 2).reshape(128, CT * N)
        ).astype(bf16)
        in_maps.append({
            "xT": xt,
            "wqT": wq_hp, "wkT": wk_hp, "wvT": wvT, "pwT": pwT, "pb": pb,
            "ones2": ones2,
        })
    return in_maps


def _run(in_maps, **kwargs):
    from concourse.bass_utils import run_bass_kernel_spmd

    return run_bass_kernel_spmd(_get_nc(), in_maps,
                                core_ids=list(range(NCORES)), **kwargs)


def _gather(res):
    out = np.stack([res.results[b]["out"].T for b in range(B)], axis=0)
    return np.ascontiguousarray(out.astype(np.float32))


def kernel(x, qkv_w, proj_w, proj_b):
    return _gather(_run(_prep_in_maps(x, qkv_w, proj_w, proj_b)))
